# revision 1
# baseline (speedup 1.0000x reference)
"""Trainium2 Bass kernel for nn_DSQGAttentionN (sparse offset-attention).

Sharding: 16 heads / 8 cores = 2 heads per core (head parallel). Each core
computes its 2 heads' attention plus a column-shard of the gate and output
projection, producing a full-shape (2048, 1024) partial output; the host sums
the 8 partials and adds the output bias.
"""

import math

import numpy as np
import ml_dtypes

BF16 = ml_dtypes.bfloat16

DY = [96, 128, 192, 256, 384, 512, 768, 1024, 1536]  # dyadic offsets
NDY = len(DY)
N, D, H, HD = 2048, 1024, 16, 64
NCH = N // 128        # 16 sequence chunks of 128
NCORES = 8
QPAD = 64             # query tail padding (dense band reads past n=2047)
KPAD = 1536           # key front padding (dyadic shifts read past n=0)
VPADC = 12            # front zero chunks on n-layout V (dyadic chunk shifts)
NDD = 9               # 8 contraction chunks of x + 1 bias (ones-row) chunk

_STATE = {}


def _build_nc(debug=False):
    import concourse.bass as bass
    import concourse.tile as tile
    import concourse.mybir as mybir
    from concourse import bacc
    from concourse.bass import ds
    from concourse.masks import make_identity
    from contextlib import ExitStack

    dt = mybir.dt
    f32, bf = dt.float32, dt.bfloat16
    AF = mybir.ActivationFunctionType
    OP = mybir.AluOpType
    AX = mybir.AxisListType

    nc = bacc.Bacc("TRN2")

    WPK = 4 * NDD * 128 + D + 2 * 2 * 192 + 2 * NDY * NCH
    xT = nc.dram_tensor("xT", (128, NDD, N), bf, kind="ExternalInput").ap()
    wpk = nc.dram_tensor("wpk", (128, WPK), bf, kind="ExternalInput").ap()
    out = nc.dram_tensor("out", (128, NCH, D), bf, kind="ExternalOutput").ap()
    dbg = {}
    if debug:
        for nm, shp, dtt in [
            ("dqt", (128, N + QPAD), bf), ("dkt", (128, KPAD + N), bf),
            ("dv2", (128, VPADC + NCH, 130), bf),
            ("ddyS", (128, 2, NDY, NCH), f32),
            ("ddyP", (128, 2, NDY, NCH), bf),
            ("ddyO", (128, NCH, 128), f32), ("dfgT", (128, NCH, 128), bf),
            ("dpt", (128, 2, NCH, 256), bf),
        ]:
            dbg[nm] = nc.dram_tensor(nm, shp, dtt, kind="ExternalOutput").ap()

    with tile.TileContext(nc) as tc, ExitStack() as ctx:
        sing = ctx.enter_context(tc.tile_pool(name="sing", bufs=1))

        # ---- resident SBUF tensors ----
        xt = sing.tile([128, NDD, N], bf)
        wpk_s = sing.tile([128, WPK], bf)
        _o = 0
        wq_s = wpk_s[:, _o : _o + NDD * 128].rearrange("p (c m) -> p c m", c=NDD)
        _o += NDD * 128
        wk_s = wpk_s[:, _o : _o + NDD * 128].rearrange("p (c m) -> p c m", c=NDD)
        _o += NDD * 128
        wv_s = wpk_s[:, _o : _o + NDD * 128].rearrange("p (c m) -> p c m", c=NDD)
        _o += NDD * 128
        wg_s = wpk_s[:, _o : _o + NDD * 128].rearrange("p (c m) -> p c m", c=NDD)
        _o += NDD * 128
        wo_s = wpk_s[:, _o : _o + D]
        _o += D
        em_s = wpk_s[:, _o : _o + 768].rearrange("p (h v j) -> p h v j", h=2, v=2)
        _o += 768
        v9_s = wpk_s[:, _o : _o + 2 * NDY * NCH].rearrange(
            "p (h i c) -> p h i c", h=2, i=NDY
        )
        scrap = sing.tile([1, 16], f32)
        scrap2 = sing.tile([1, 16], f32)

        qt = sing.tile([128, N + QPAD], bf)        # Q^T  [hd2, n] (scaled)
        kt = sing.tile([128, KPAD + N], bf)        # K^T  [hd2, kpad + n]
        vt = sing.tile([128, N], bf)               # V^T  [hd2, n]
        gt = sing.tile([128, N], bf)               # gate^T [g, n] (sigmoided)
        v2 = sing.tile([128, VPADC + NCH, 130], bf)   # V [n128, c, hd2+ones]
        v96 = sing.tile([128, NCH, 130], bf)          # V shifted by 96
        v192 = sing.tile([128, NCH, 130], bf)         # V shifted by 192
        ptall = sing.tile([128, 2, NCH, 256], bf)     # exp'd masked P^T tiles
        dyS = sing.tile([128, 2, NDY, NCH], f32)      # dyadic scores [n128, h, dy, c]
        dyP = sing.tile([128, 2, NDY, NCH], bf)       # exp'd masked dyadic
        dyA = sing.tile([128, NCH, 128], bf)          # dyadic AV accum (DVE)
        dyB = sing.tile([128, NCH, 128], bf)          # dyadic AV accum (POOL)
        zdy = sing.tile([128, 2, NCH], f32)           # dyadic z
        fgT = sing.tile([128, NCH, 128], bf)          # (o*g/z)^T [g, c, n128]
        outsb = sing.tile([128, NCH, D], bf)          # output staging
        ones1 = sing.tile([128, 1], bf)
        ones2 = sing.tile([128, 2], bf)
        ident = sing.tile([128, 128], bf)

        nc.vector.memset(scrap2, 0.0)

        # HWDGE semaphore-lane tracking (see hw_join)
        hwd = {"rr": 0, "last": {}}

        def hw_dma(out_ap, in_ap):
            dref = nc.sync.dma_start(out=out_ap, in_=in_ap)
            hwd["last"][hwd["rr"] % 8] = dref
            hwd["rr"] += 1
            return dref

        def hw_join(consumer_engine):
            joins = []
            for half in (range(0, 4), range(4, 8)):
                if consumer_engine == "act":
                    j = nc.scalar.copy(scrap, scrap2)
                else:
                    j = nc.vector.tensor_copy(scrap, scrap2)
                for ln in half:
                    if ln in hwd["last"]:
                        tile.add_dep_helper(
                            j.ins, hwd["last"][ln].ins, sync=True,
                            reason="hwdge join",
                        )
                joins.append(j)
            return joins

        # ---- load inputs ----
        hw_dma(xt[:, 0:3, :], xT[:, 0:3, :])
        hw_dma(xt[:, 3:6, :], xT[:, 3:6, :])
        hw_dma(xt[:, 6:NDD, :], xT[:, 6:NDD, :])
        hw_dma(wpk_s, wpk)

        nc.vector.memset(ones1, 1.0)
        nc.vector.memset(ones2, 0.0)
        nc.vector.memset(ones2[0:64, 0:1], 1.0)
        nc.vector.memset(ones2[64:128, 1:2], 1.0)
        make_identity(nc, ident)
        nc.vector.memset(qt[:, N : N + QPAD], 0.0)
        nc.vector.memset(kt[:, 0:KPAD], 0.0)
        nc.gpsimd.memset(v2[:, 0:VPADC, :], 0.0)
        nc.gpsimd.memset(ptall, 0.0)
        # ones columns for the fused softmax-denominator matmul
        nc.gpsimd.memset(v2[:, VPADC:, 64:65], 1.0)
        nc.gpsimd.memset(v2[:, VPADC:, 129:130], 1.0)

        # ---- projections: out^T = W_pack.T @ x^T, biases via ones-row ----
        with ExitStack() as pctx:
            psP = pctx.enter_context(tc.tile_pool(name="psP", bufs=3, space="PSUM"))
            psT = pctx.enter_context(tc.tile_pool(name="psT", bufs=2, space="PSUM"))

            def proj(w_sb, dst, func, dst_off=0):
                for nb in range(N // 512):
                    ps = psP.tile([128, 512], f32)
                    for c in range(NDD):
                        nc.tensor.matmul(
                            ps,
                            w_sb[:, c, :],
                            xt[:, c, ds(nb * 512, 512)],
                            start=(c == 0),
                            stop=(c == NDD - 1),
                        )
                    nc.scalar.activation(
                        dst[:, ds(dst_off + nb * 512, 512)], ps, func
                    )

            proj(wq_s, qt, AF.Copy)
            proj(wk_s, kt, AF.Copy, dst_off=KPAD)
            proj(wv_s, vt, AF.Copy)
            proj(wg_s, gt, AF.Sigmoid)

            # transpose V^T into n-on-partition layout (cols 0:64 h0, 65:129 h1)
            for c in range(NCH):
                tp = psT.tile([128, 128], bf)
                nc.tensor.transpose(tp, vt[:, ds(c * 128, 128)], ident)
                if c % 2 == 0:
                    nc.scalar.copy(v2[:, VPADC + c, 0:64], tp[:, 0:64])
                    nc.scalar.copy(v2[:, VPADC + c, 65:129], tp[:, 64:128])
                else:
                    nc.vector.tensor_copy(v2[:, VPADC + c, 0:64], tp[:, 0:64])
                    nc.scalar.copy(v2[:, VPADC + c, 65:129], tp[:, 64:128])

        # shifted copies of V for the two non-multiple-of-128 dyadic offsets
        hw_dma(v96[0:96, :, :], v2[32:128, VPADC - 1 : VPADC + 15, :])
        hw_dma(v96[96:128, :, :], v2[0:32, VPADC : VPADC + 16, :])
        hw_dma(v192[0:64, :, :], v2[64:128, VPADC - 2 : VPADC + 14, :])
        hw_dma(v192[64:128, :, :], v2[0:64, VPADC - 1 : VPADC + 15, :])

        # ---- dense pass 1: scores -> exp -> mask, kept resident ----
        with ExitStack() as actx:
            psS = actx.enter_context(tc.tile_pool(name="psS", bufs=4, space="PSUM"))
            for h in range(2):
                hp = ds(64 * h, 64)
                for kc in range(NCH):
                    span = 192 if kc < NCH - 1 else 128
                    st = psS.tile([128, 192], f32, tag="st", name="st")
                    nc.tensor.matmul(
                        st[:, 0:span],
                        kt[hp, ds(KPAD + kc * 128, 128)],
                        qt[hp, ds(kc * 128, span)],
                        start=True,
                        stop=True,
                    )
                    pt = ptall[:, h, kc, :]
                    nc.scalar.activation(pt[0:128, 0:span], st[:, 0:span], AF.Exp)
                    meng = nc.vector if kc % 2 == 0 else nc.gpsimd
                    meng.tensor_mul(
                        pt[0:128, 0:span],
                        pt[0:128, 0:span],
                        em_s[:, h, 1 if kc == NCH - 1 else 0, 0:span],
                    )

        # ---- dyadic offsets: scores ----
        with ExitStack() as dctx:
            psD = dctx.enter_context(tc.tile_pool(name="psD", bufs=4, space="PSUM"))
            sbp2 = dctx.enter_context(tc.tile_pool(name="sbp2", bufs=3))
            for i, d in enumerate(DY):
                prod = sbp2.tile([128, N], bf, tag="prod")
                peng = nc.gpsimd if i % 2 == 0 else nc.vector
                peng.tensor_mul(prod, qt[:, 0:N], kt[:, KPAD - d : KPAD - d + N])
                drs = sbp2.tile([2, N], f32, tag="drs", name="drs")
                for b in range(4):
                    drb = psD.tile([2, 512], f32, tag="dr", name="drb")
                    nc.tensor.matmul(
                        drb, ones2, prod[:, ds(b * 512, 512)], start=True, stop=True
                    )
                    ddst = bass.AP(
                        tensor=drs.tensor,
                        offset=drs.offset + 4 * b,
                        ap=[list(drs.ap[0]), [NCH, 128], [1, 4]],
                    )
                    dsrc = bass.AP(
                        tensor=drb.tensor,
                        offset=drb.offset,
                        ap=[list(drb.ap[0]), [1, 128], [128, 4]],
                    )
                    if (i + b) % 3 != 2:
                        nc.scalar.copy(ddst, dsrc)
                    else:
                        nc.vector.tensor_copy(ddst, dsrc)
                for h in range(2):
                    ssl = drs[h : h + 1, :]
                    s_ap = bass.AP(
                        tensor=ssl.tensor,
                        offset=ssl.offset,
                        ap=[list(ssl.ap[0]), [NCH, 128], [1, NCH]],
                    )
                    dsl = dyS[:, h, i, :]
                    d_ap = bass.AP(
                        tensor=dsl.tensor,
                        offset=dsl.offset,
                        ap=[list(dsl.ap[0]), [1, NCH]],
                    )
                    hw_dma(d_ap, s_ap)

        # ---- dyadic softmax pieces ----
        import concourse.bass as _b

        _ejs = hw_join("act")
        _exp_i = nc.scalar.activation(dyP, dyS, AF.Exp)
        for _j in _ejs:
            tile.add_dep_helper(_exp_i.ins, _j.ins, sync=False, reason="join order")
        nc.vector.tensor_mul(dyP, dyP, v9_s)
        nc.vector.tensor_reduce(
            zdy, dyP.rearrange("p h i c -> p h c i"), AX.X, OP.add
        )

        # ---- dyadic AV: dyA/dyB += alpha_d * V[n - d] ----
        def dy_src(i):
            d = DY[i]
            if d == 96:
                return v96[:, 0:NCH, :]
            if d == 192:
                return v192[:, 0:NCH, :]
            m = d // 128
            return v2[:, VPADC - m : VPADC - m + NCH, :]

        def dy_src_view(i):
            a = dy_src(i)
            return _b.AP(
                tensor=a.tensor,
                offset=a.offset,
                ap=[list(a.ap[0]), [130, NCH], [65, 2], [1, 64]],
            )

        def bcast_alpha(i):
            sl = dyP[:, :, i, :]  # (128, 2, NCH)
            return _b.AP(
                tensor=sl.tensor,
                offset=sl.offset,
                ap=[list(sl.ap[0]), list(sl.ap[2]), list(sl.ap[1]), [0, 64]],
            )

        def acc_view(acc):
            return acc.rearrange("p c (h e) -> p c h e", h=2)

        sbt = ctx.enter_context(tc.tile_pool(name="sbt", bufs=3))
        dve_set = [0, 2, 4, 6, 8, 7]  # 6 on DVE, 3 on Pool
        GQ = 4  # chunks per dyadic-AV group

        def gsl(ap4, g):
            # slice group g of a (128, NCH, 2, 64)-shaped view
            return ap4[:, g * GQ : (g + 1) * GQ, :, :]

        for g in range(NCH // GQ):
            first_a, first_b = True, True
            for i in range(NDY):
                eng = nc.vector if i in dve_set else nc.gpsimd
                acc = dyA if i in dve_set else dyB
                accv = gsl(acc_view(acc), g)
                srcv = gsl(dy_src_view(i), g)
                alv = gsl(bcast_alpha(i), g)
                if (i in dve_set and first_a) or (i not in dve_set and first_b):
                    eng.tensor_mul(accv, srcv, alv)
                    if i in dve_set:
                        first_a = False
                    else:
                        first_b = False
                else:
                    tmp = sbt.tile([128, GQ, 128], bf, tag=f"tmp{i % 2}")
                    eng.tensor_mul(acc_view(tmp), srcv, alv)
                    eng.tensor_add(accv, accv, acc_view(tmp))

        # ---- dense pass 2 + tail, fused per query chunk ----
        with ExitStack() as octx:
            psO = octx.enter_context(tc.tile_pool(name="psO", bufs=3, space="PSUM"))
            psF = octx.enter_context(tc.tile_pool(name="psF", bufs=2, space="PSUM"))
            psQ = octx.enter_context(tc.tile_pool(name="psQ", bufs=3, space="PSUM"))
            sbf = octx.enter_context(tc.tile_pool(name="sbf", bufs=4))
            for qc in range(NCH):
                od = psO.tile([128, 2, 65], f32, tag="od", name="od")
                for h in range(2):
                    first = True
                    if qc > 0:
                        nc.tensor.matmul(
                            od[:, h, :],
                            ptall[:, h, qc - 1, 128:256],
                            v2[:, VPADC + qc - 1, ds(65 * h, 65)],
                            start=True,
                            stop=False,
                            skip_group_check=True,
                        )
                        first = False
                    nc.tensor.matmul(
                        od[:, h, :],
                        ptall[:, h, qc, 0:128],
                        v2[:, VPADC + qc, ds(65 * h, 65)],
                        start=first,
                        stop=True,
                        skip_group_check=True,
                    )
                # z and reciprocal
                zt = sbf.tile([128, 2], f32, tag="zt", name="zt")
                nc.vector.tensor_add(
                    zt, od[:, :, 64], zdy[:, :, qc]
                )
                rzt = sbf.tile([128, 2], f32, tag="rzt", name="rzt")
                nc.vector.reciprocal(rzt, zt)
                # a = (dense + dyadic) * rz
                a_t = sbf.tile([128, 2, 64], bf, tag="a", name="a")
                nc.vector.tensor_add(
                    a_t, od[:, :, 0:64],
                    dyA[:, qc, :].rearrange("p (h e) -> p h e", h=2),
                )
                nc.vector.tensor_add(
                    a_t, a_t,
                    dyB[:, qc, :].rearrange("p (h e) -> p h e", h=2),
                )
                for h in range(2):
                    nc.vector.tensor_scalar_mul(
                        a_t[:, h, :], a_t[:, h, :], rzt[:, h : h + 1]
                    )
                tp = psF.tile([128, 128], bf, name="tp")
                nc.tensor.transpose(tp, a_t.rearrange("p h e -> p (h e)"), ident)
                nc.vector.tensor_mul(
                    fgT[:, qc, :], tp, gt[:, ds(qc * 128, 128)]
                )

            for qc in range(NCH):
                for jb in range(2):
                    po = psQ.tile([128, 512], f32, tag="po", name="po")
                    nc.tensor.matmul(
                        po,
                        fgT[:, qc, :],
                        wo_s[:, ds(jb * 512, 512)],
                        start=True,
                        stop=True,
                    )
                    if (qc + jb) % 2 == 0:
                        nc.scalar.copy(outsb[:, qc, ds(jb * 512, 512)], po)
                    else:
                        nc.vector.tensor_copy(
                            outsb[:, qc, ds(jb * 512, 512)], po
                        )
            for g in range(4):
                hw_dma(out[:, 4 * g : 4 * g + 4, :], outsb[:, 4 * g : 4 * g + 4, :])

        if debug:
            for nm, sb in [
                ("dqt", qt), ("dkt", kt), ("dv2", v2[:, :, :]),
                ("ddyS", dyS), ("ddyP", dyP),
                ("ddyO", dyA), ("dfgT", fgT), ("dpt", ptall),
            ]:
                if nm in dbg:
                    nc.sync.dma_start(out=dbg[nm], in_=sb)

    nc.compile()
    return nc


def _host_prep(x, Wqkv, bqkv, Wgate, bgate, Wout, bout, pos_bias):
    x2 = np.asarray(x, np.float32).reshape(N, D)
    xTc = np.ascontiguousarray(x2.T).reshape(8, 128, N).transpose(1, 0, 2)
    ones_ch = np.zeros((128, 1, N), np.float32)
    ones_ch[0] = 1.0
    xT = np.ascontiguousarray(
        np.concatenate([xTc, ones_ch], axis=1)
    ).astype(BF16)

    def wpack(W, b):
        # -> (128, 9, 128): [p, c, m] = W[m, 128c+p]; bias in chunk 8 row p=0
        Wt = np.ascontiguousarray(W.T).reshape(8, 128, 128).transpose(1, 0, 2)
        bc = np.zeros((128, 1, 128), np.float32)
        bc[0, 0] = b
        return np.ascontiguousarray(np.concatenate([Wt, bc], axis=1)).astype(BF16)

    scale = HD ** -0.5
    in_maps = []
    for cid in range(NCORES):
        r0 = 128 * cid
        wq = wpack(Wqkv[r0 : r0 + 128] * scale, bqkv[r0 : r0 + 128] * scale)
        wk = wpack(Wqkv[D + r0 : D + r0 + 128], bqkv[D + r0 : D + r0 + 128])
        wv = wpack(Wqkv[2 * D + r0 : 2 * D + r0 + 128], bqkv[2 * D + r0 : 2 * D + r0 + 128])
        wgp = wpack(Wgate[r0 : r0 + 128], bgate[r0 : r0 + 128])
        wop = np.ascontiguousarray(Wout[:, r0 : r0 + 128].T).astype(BF16)

        emt = np.zeros((128, 2, 2, 192), np.float32)
        p = np.arange(128)[:, None]
        j = np.arange(192)[None, :]
        o = j - p  # offset = query - key
        band = (o >= 0) & (o <= 64)
        for h in range(2):
            eb = np.exp(pos_bias[:65, 2 * cid + h]).astype(np.float32)
            vals = np.where(band, eb[np.clip(o, 0, 64)], 0.0)
            emt[:, h, 0, :] = vals
            emt[:, h, 1, :] = np.where(j < 128, vals, 0.0)
        emt = emt.astype(BF16)

        OFFS = sorted(set(range(65)) | set(DY))
        nidx = np.arange(128)[:, None, None, None] + 128 * np.arange(NCH)[None, None, None, :]
        dys = np.array(DY)[None, None, :, None]
        valid = (nidx >= dys).astype(np.float32)  # (128, 1, NDY, NCH)
        eb9 = np.zeros((1, 2, NDY, 1), np.float32)
        for h in range(2):
            for i, d in enumerate(DY):
                eb9[0, h, i, 0] = math.exp(pos_bias[OFFS.index(d), 2 * cid + h])
        v9t = (valid * eb9).astype(BF16)  # (128, 2, NDY, NCH)

        wpk = np.concatenate(
            [
                wq.reshape(128, -1),
                wk.reshape(128, -1),
                wv.reshape(128, -1),
                wgp.reshape(128, -1),
                wop.reshape(128, -1),
                emt.reshape(128, -1),
                v9t.reshape(128, -1),
            ],
            axis=1,
        )
        in_maps.append({"xT": xT, "wpk": np.ascontiguousarray(wpk)})
    return in_maps


def kernel(x, Wqkv, bqkv, Wgate, bgate, Wout, bout, pos_bias, offsets=None, **kw):
    x = np.asarray(x, np.float32)
    Wqkv = np.asarray(Wqkv, np.float32)
    bqkv = np.asarray(bqkv, np.float32)
    Wgate = np.asarray(Wgate, np.float32)
    bgate = np.asarray(bgate, np.float32)
    Wout = np.asarray(Wout, np.float32)
    bout = np.asarray(bout, np.float32)
    pos_bias = np.asarray(pos_bias, np.float32)

    if "nc" not in _STATE:
        _STATE["nc"] = _build_nc()
    nc = _STATE["nc"]

    in_maps = _host_prep(x, Wqkv, bqkv, Wgate, bgate, Wout, bout, pos_bias)

    from concourse.bass_utils import run_bass_kernel_spmd

    res = run_bass_kernel_spmd(
        nc, in_maps, core_ids=list(range(NCORES)), **_STATE.get("run_kwargs", {})
    )
    _STATE["last"] = res

    total = np.zeros((N, D), np.float32)
    for r in res.results:
        total += r["out"].astype(np.float32).transpose(1, 0, 2).reshape(N, D)
    total += bout
    return total.reshape(1, N, D).astype(np.float32)



# revision 17
# speedup vs baseline: 1.6909x; 1.6909x over previous
"""Trainium2 Bass kernel for nn_DSQGAttentionN (sparse offset-attention).

Sharding: 16 heads / 8 cores = 2 heads per core (head parallel). Each core
computes its 2 heads' attention plus a column-shard of the gate and output
projection, producing a full-shape (2048, 1024) partial output; the host sums
the 8 partials and adds the output bias.

Design notes (v2):
- Dense band widened to offsets 0..192 so the 96/128/192 dyadic offsets ride
  the band matmuls; only 6 chunk-aligned dyadic offsets (256..1536) remain.
- Dyadic scores: elementwise q*k product (DVE), then per-chunk ones-matmuls
  on PE reduce the head dim -> scores land directly in n-on-partition layout
  (no scatter DMAs, no 2-partition copies).
- V / gate / Wout packed with (e,h)-interleaved feature order so dyadic-AV
  broadcast views keep a packed last dim (DVE 2x mode); the band-AV matmul
  reads V through a stride-2 column view whose 65th element is the ones col.
- Gate sigmoid via tanh (shares the exp activation table): no table reloads.
- QK projection is chunk-outer over 8 PSUM banks so PE streams behind the
  per-chunk x DMAs; biases fold into the PSUM->SBUF drains.
- Output projection matmuls write bf16 PSUM tiles DMA'd straight to DRAM.
"""

import math
import os

import numpy as np
import ml_dtypes

_VAR = os.environ.get("KVAR", "")

BF16 = ml_dtypes.bfloat16

N, D, H, HD = 2048, 1024, 16, 64
NCH = 16              # sequence chunks of 128
NXC = 8               # contraction chunks of x
W = 320               # band window per key chunk (128 keys + 192 query overhang)
KPAD = 1536           # key front padding for dyadic shifted reads
VPADC = 12            # front zero chunks on n-layout V
DY6 = [256, 384, 512, 768, 1024, 1536]
NDY = 6
NCORES = 8

_DENSE_LOCAL_W = 64
_DYADIC = [96, 128, 192, 256, 384, 512, 768, 1024, 1536]
OFFSETS = sorted(set(range(0, _DENSE_LOCAL_W + 1)) | set(_DYADIC))
BAND_OFFS = sorted(set(range(0, 65)) | {96, 128, 192})

# wpk column layout (bf16)
_WQ = 0
_WK = _WQ + NXC * 128
_WV = _WK + NXC * 128
_WG = _WV + NXC * 128
_WO = _WG + NXC * 128
_EM = _WO + D                     # 2 * W
_V9 = _EM + 2 * W                 # NCH * NDY * 2
WPK = _V9 + NCH * NDY * 2

_STATE = {}


def _build_nc(debug=False):
    import concourse.bass as bass
    import concourse.tile as tile
    import concourse.mybir as mybir
    from concourse import bacc
    from concourse.bass import ds
    from concourse.masks import make_identity
    from contextlib import ExitStack

    dt = mybir.dt
    f32, bf = dt.float32, dt.bfloat16
    AF = mybir.ActivationFunctionType
    OP = mybir.AluOpType
    AX = mybir.AxisListType

    nc = bacc.Bacc("TRN2")

    xT = nc.dram_tensor("xT", (128, NXC, N), bf, kind="ExternalInput").ap()
    wpk = nc.dram_tensor("wpk", (128, WPK), bf, kind="ExternalInput").ap()
    bias4 = nc.dram_tensor("bias4", (128, 4), f32, kind="ExternalInput").ap()
    out = nc.dram_tensor("out", (128, NCH, D), bf, kind="ExternalOutput").ap()
    dbg = {}
    if debug:
        for nm, shp, dtt in [
            ("dqt", (128, N), bf), ("dkt", (128, KPAD + N), bf),
            ("dvt", (128, N), bf), ("dgt", (128, N), bf),
            ("dv2", (128, VPADC + NCH, 130), bf),
            ("ddyP", (128, NCH, NDY, 2), bf),
            ("dzdy", (128, NCH, 2), f32),
            ("ddyA", (128, NCH, 128), bf), ("ddyB", (128, NCH, 128), bf),
            ("dfgT", (128, NCH, 128), bf),
            ("dpt", (128, 2, NCH, W), bf),
        ]:
            dbg[nm] = nc.dram_tensor(nm, shp, dtt, kind="ExternalOutput").ap()

    with tile.TileContext(nc) as tc, ExitStack() as ctx:
        sing = ctx.enter_context(tc.tile_pool(name="sing", bufs=1))

        # ---- resident SBUF tensors ----
        xt = sing.tile([128, NXC, N], bf)
        wpk_s = sing.tile([128, WPK], bf)
        wq_s = wpk_s[:, _WQ:_WK].rearrange("p (c m) -> p c m", c=NXC)
        wk_s = wpk_s[:, _WK:_WV].rearrange("p (c m) -> p c m", c=NXC)
        wv_s = wpk_s[:, _WV:_WG].rearrange("p (c m) -> p c m", c=NXC)
        wg_s = wpk_s[:, _WG:_WO].rearrange("p (c m) -> p c m", c=NXC)
        wo_s = wpk_s[:, _WO:_EM]
        em_s = wpk_s[:, _EM:_V9].rearrange("p (h j) -> p h j", h=2)
        v9_s = wpk_s[:, _V9:WPK].rearrange("p (c i h) -> p c i h", c=NCH, i=NDY)

        qt = sing.tile([128, N], bf)               # Q^T [m, n] (scaled)
        kt = sing.tile([128, KPAD + N], bf)        # K^T [m, kpad + n]
        vt = sing.tile([128, N], bf)               # V^T [j(interleaved), n]
        gt = sing.tile([128, N], bf)               # gate^T [j, n] (sigmoided)
        v2i = sing.tile([128, VPADC + NCH, 130], bf)   # V [n128, c, j + 2 ones]
        ptall = sing.tile([128, 2, NCH, W], bf)    # masked exp'd band P^T
        dyPh = sing.tile([128, NCH, NDY, 2], bf)   # exp'd masked dyadic alpha
        zdy = sing.tile([128, NCH, 2], f32)        # dyadic z
        dyA = sing.tile([128, NCH, 128], bf)       # dyadic AV accum (DVE)
        dyB = sing.tile([128, NCH, 128], bf)       # dyadic AV accum (Pool)
        fgT = sing.tile([128, NCH, 128], bf)       # (o*g/z)^T [j, c, n128]
        outsb = sing.tile([128, NCH, D], bf)       # output staging
        hmask = sing.tile([128, 2], bf)
        bias_s = sing.tile([128, 4], f32)
        ident = sing.tile([128, 128], bf)

        bq_c = bias_s[:, 0:1]
        bk_c = bias_s[:, 1:2]
        bv_c = bias_s[:, 2:3]
        bg_c = bias_s[:, 3:4]

        def ap_of(t, extra, off_elems=0):
            return bass.AP(
                tensor=t.tensor, offset=t.offset + off_elems,
                ap=[list(t.ap[0])] + extra,
            )

        # ---- memsets (no input deps; run during input DMA) ----
        nc.vector.memset(kt[:, 0:KPAD], 0.0)
        nc.vector.memset(hmask, 0.0)
        nc.vector.memset(hmask[0:64, 0:1], 1.0)
        nc.vector.memset(hmask[64:128, 1:2], 1.0)
        nc.gpsimd.memset(v2i[:, 0:VPADC, :], 0.0)
        nc.gpsimd.memset(ap_of(v2i, [[130, VPADC + NCH], [1, 2]], 128), 1.0)
        make_identity(nc, ident)

        # ---- input DMAs (tile auto-wires DMA->consumer deps) ----
        nc.sync.dma_start(out=bias_s, in_=bias4)
        nc.sync.dma_start(out=wpk_s[:, _WQ:_WV], in_=wpk[:, _WQ:_WV])
        for c in range(NXC):
            nc.sync.dma_start(out=xt[:, c, :], in_=xT[:, c, :])
        nc.sync.dma_start(out=wpk_s[:, _WV:_WO], in_=wpk[:, _WV:_WO])
        nc.sync.dma_start(out=wpk_s[:, _WO:WPK], in_=wpk[:, _WO:WPK])

        # ---- QK projection: chunk-outer over 8 PSUM banks ----
        with ExitStack() as pctx:
            psQK = pctx.enter_context(
                tc.tile_pool(name="psQK", bufs=8, space="PSUM")
            )
            qk_ps = [
                psQK.tile([128, 512], f32, tag="qk", name=f"qk{i}")
                for i in range(8)
            ]
            for c in range(NXC):
                for pj, w_sb in ((0, wq_s), (1, wk_s)):
                    for b in range(4):
                        nc.tensor.matmul(
                            qk_ps[pj * 4 + b],
                            w_sb[:, c, :],
                            xt[:, c, ds(b * 512, 512)],
                            start=(c == 0),
                            stop=(c == NXC - 1),
                        )
            # drains with fused bias, split across engines (Pool cannot
            # read PSUM)
            nc.vector.tensor_scalar_add(kt[:, ds(KPAD, 512)], qk_ps[4], bk_c)
            nc.scalar.activation(qt[:, ds(0, 512)], qk_ps[0], AF.Identity, bias=bq_c)
            nc.vector.tensor_scalar_add(kt[:, ds(KPAD + 512, 512)], qk_ps[5], bk_c)
            nc.scalar.activation(qt[:, ds(512, 512)], qk_ps[1], AF.Identity, bias=bq_c)
            nc.vector.tensor_scalar_add(qt[:, ds(1024, 512)], qk_ps[2], bq_c)
            nc.scalar.activation(
                kt[:, ds(KPAD + 1024, 512)], qk_ps[6], AF.Identity, bias=bk_c
            )
            nc.vector.tensor_scalar_add(qt[:, ds(1536, 512)], qk_ps[3], bq_c)
            nc.scalar.activation(
                kt[:, ds(KPAD + 1536, 512)], qk_ps[7], AF.Identity, bias=bk_c
            )

        # ---- middle phase: VG proj + band scores + dyadic, interleaved ----
        # dyadic prods emitted in descending-d order: large shifts only need
        # early kt blocks, so they unblock as soon as those drains land.
        PRODS = list(range(NDY - 1, -1, -1))  # i = 5, 4, 3, 2, 1, 0

        with ExitStack() as mctx:
            psVG = mctx.enter_context(
                tc.tile_pool(name="psVG", bufs=3, space="PSUM")
            )
            psS = mctx.enter_context(tc.tile_pool(name="psS", bufs=2, space="PSUM"))
            psDY = mctx.enter_context(
                tc.tile_pool(name="psDY", bufs=1, space="PSUM")
            )
            psT = mctx.enter_context(tc.tile_pool(name="psT", bufs=2, space="PSUM"))
            sbp = mctx.enter_context(tc.tile_pool(name="sbp", bufs=2))

            dy_ps = psDY.tile([128, NCH, NDY * 2], f32, name="dy_ps")

            def vg_subphase(w_sb, blocks):
                tiles = []
                for b in blocks:
                    ps = psVG.tile([128, 512], f32, tag="vg", name="vg")
                    for c in range(NXC):
                        nc.tensor.matmul(
                            ps, w_sb[:, c, :], xt[:, c, ds(b * 512, 512)],
                            start=(c == 0), stop=(c == NXC - 1),
                        )
                    tiles.append(ps)
                return tiles

            def band_kc(kc):
                span = min(W, N - 128 * kc)
                for hl in range(2):
                    st = psS.tile([128, W], f32, tag="st", name="st", bufs=2)
                    hp = ds(64 * hl, 64)
                    nc.tensor.matmul(
                        st[:, 0:span],
                        kt[hp, ds(KPAD + kc * 128, 128)],
                        qt[hp, ds(kc * 128, span)],
                        start=True, stop=True,
                    )
                    pt = ptall[:, hl, kc, :]
                    nc.scalar.activation(pt[0:128, 0:span], st[:, 0:span], AF.Exp)

            def band_mask(kc, hl, eng):
                span = min(W, N - 128 * kc)
                pt = ptall[:, hl, kc, :]
                eng.tensor_mul(
                    pt[0:128, 0:span], pt[0:128, 0:span], em_s[:, hl, 0:span]
                )

            prod_tiles = {}

            def emit_prod(i):
                prod = sbp.tile(
                    [128, N], bf, tag=f"prod{i}", name="prod", bufs=1
                )
                nc.vector.tensor_mul(prod, qt, kt[:, ds(KPAD - DY6[i], N)])
                prod_tiles[i] = prod

            def emit_reduce(i):
                for c in range(NCH):
                    o1 = bass.AP(
                        tensor=dy_ps.tensor,
                        offset=dy_ps.offset + c * NDY * 2 + i * 2,
                        ap=[list(dy_ps.ap[0]), [1, 2]],
                    )
                    nc.tensor.matmul(
                        o1,
                        prod_tiles[i][:, ds(c * 128, 128)],
                        hmask,
                        start=True, stop=True,
                        skip_group_check=True,
                    )

            def v_transpose(c):
                tp = psT.tile([128, 128], bf, tag="tp", name="tp", bufs=2)
                nc.tensor.transpose(tp, vt[:, ds(c * 128, 128)], ident)
                nc.scalar.copy(v2i[:, VPADC + c, 0:128], tp)

            # --- V blocks 0,1 ---
            vg_t = vg_subphase(wv_s, [0, 1])
            nc.scalar.activation(vt[:, ds(0, 512)], vg_t[0], AF.Identity, bias=bv_c)
            nc.vector.tensor_scalar_add(vt[:, ds(512, 512)], vg_t[1], bv_c)
            emit_prod(5)
            emit_prod(4)
            emit_reduce(5)
            for kc in range(0, 4):
                band_kc(kc)
            emit_prod(3)
            for kc in range(0, 4):
                band_mask(kc, 0, nc.vector)
                band_mask(kc, 1, nc.gpsimd)

            # --- V blocks 2,3 ---
            vg_t = vg_subphase(wv_s, [2, 3])
            nc.scalar.activation(
                vt[:, ds(1024, 512)], vg_t[0], AF.Identity, bias=bv_c
            )
            nc.vector.tensor_scalar_add(vt[:, ds(1536, 512)], vg_t[1], bv_c)
            emit_prod(2)
            emit_reduce(4)
            for kc in range(4, 8):
                band_kc(kc)
            emit_prod(1)
            for kc in range(4, 8):
                band_mask(kc, 0, nc.vector)
                band_mask(kc, 1, nc.gpsimd)
            for c in range(0, 8):
                v_transpose(c)

            # --- G blocks 0,1 ---
            vg_t = vg_subphase(wg_s, [0, 1])
            nc.scalar.activation(
                gt[:, ds(0, 512)], vg_t[0], AF.Tanh, bias=bg_c, scale=0.5
            )
            nc.scalar.activation(
                gt[:, ds(512, 512)], vg_t[1], AF.Tanh, bias=bg_c, scale=0.5
            )
            emit_prod(0)
            emit_reduce(3)
            emit_reduce(2)
            for kc in range(8, 12):
                band_kc(kc)
            for kc in range(8, 12):
                band_mask(kc, 0, nc.vector)
                band_mask(kc, 1, nc.gpsimd)

            # --- G blocks 2,3 ---
            vg_t = vg_subphase(wg_s, [2, 3])
            nc.scalar.activation(
                gt[:, ds(1024, 512)], vg_t[0], AF.Tanh, bias=bg_c, scale=0.5
            )
            nc.scalar.activation(
                gt[:, ds(1536, 512)], vg_t[1], AF.Tanh, bias=bg_c, scale=0.5
            )
            emit_reduce(1)
            emit_reduce(0)
            for kc in range(12, NCH):
                band_kc(kc)
            for c in range(8, NCH):
                v_transpose(c)
            for kc in range(12, NCH):
                band_mask(kc, 0, nc.vector)
                band_mask(kc, 1, nc.gpsimd)
            # gate affine: sigmoid = 0.5*tanh + 0.5
            for b in range(4):
                nc.vector.tensor_scalar(
                    gt[:, ds(b * 512, 512)], gt[:, ds(b * 512, 512)],
                    0.5, 0.5, OP.mult, OP.add,
                )

            # ---- dyadic: exp -> mask -> z -> AV, per 4-chunk group ----
            for g in range(4):
                g4 = ds(g * 4, 4)
                nc.scalar.activation(
                    dyPh[:, g4, :, :], dy_ps[:, g4, :], AF.Exp
                )
                nc.vector.tensor_mul(
                    dyPh[:, g4, :, :], dyPh[:, g4, :, :], v9_s[:, g4, :, :]
                )
                nc.vector.tensor_reduce(
                    zdy[:, g4, :],
                    dyPh[:, g4, :, :].rearrange("p c i h -> p c h i"),
                    AX.X, OP.add,
                )

                def acc_v(t):
                    return ap_of(t, [[128, 4], [2, 64], [1, 2]], g * 4 * 128)

                def src_v(i):
                    m = DY6[i] // 128
                    return ap_of(
                        v2i, [[130, 4], [2, 64], [1, 2]],
                        (VPADC - m + g * 4) * 130,
                    )

                def alpha_v(i):
                    return ap_of(
                        dyPh, [[NDY * 2, 4], [0, 64], [1, 2]],
                        g * 4 * NDY * 2 + i * 2,
                    )

                nc.gpsimd.tensor_mul(acc_v(dyB), src_v(5), alpha_v(5))
                tmpb = sbp.tile([128, 4, 128], bf, tag="avtb", name="avtb", bufs=2)
                tvb = ap_of(tmpb, [[128, 4], [2, 64], [1, 2]])
                nc.gpsimd.tensor_mul(tvb, src_v(4), alpha_v(4))
                nc.gpsimd.tensor_add(acc_v(dyB), acc_v(dyB), tvb)
                nc.vector.tensor_mul(acc_v(dyA), src_v(0), alpha_v(0))
                for i in range(1, 4):
                    tmp = sbp.tile(
                        [128, 4, 128], bf, tag="avt", name="avt", bufs=2
                    )
                    tv = ap_of(tmp, [[128, 4], [2, 64], [1, 2]])
                    nc.vector.tensor_mul(tv, src_v(i), alpha_v(i))
                    nc.vector.tensor_add(acc_v(dyA), acc_v(dyA), tv)

        # ---- pass 2: per query chunk (software-pipelined od by 1) ----
        with ExitStack() as octx:
            psO = octx.enter_context(tc.tile_pool(name="psO", bufs=3, space="PSUM"))
            psF = octx.enter_context(tc.tile_pool(name="psF", bufs=2, space="PSUM"))
            psQ = octx.enter_context(tc.tile_pool(name="psQ", bufs=3, space="PSUM"))
            sbf = octx.enter_context(tc.tile_pool(name="sbf", bufs=4))
            od_tiles = {}

            def emit_od(qc):
                od = psO.tile([128, 2, 65], f32, tag="od", name="od")
                for hl in range(2):
                    if "B" in _VAR:
                        mv = ap_of(v2i, [[1, 65]], (VPADC + qc) * 130)
                    else:
                        mv = ap_of(v2i, [[2, 65]], (VPADC + qc) * 130 + hl)
                    nc.tensor.matmul(
                        od[:, hl, :], ptall[:, hl, qc, 0:128], mv,
                        start=True, stop=(qc == 0),
                        skip_group_check=True,
                    )
                    if qc >= 1:
                        mv = (ap_of(v2i, [[1, 65]], (VPADC + qc - 1) * 130)
                              if "B" in _VAR else
                              ap_of(v2i, [[2, 65]], (VPADC + qc - 1) * 130 + hl))
                        nc.tensor.matmul(
                            od[:, hl, :], ptall[:, hl, qc - 1, 128:256], mv,
                            start=False, stop=(qc == 1),
                            skip_group_check=True,
                        )
                    if qc >= 2:
                        mv = (ap_of(v2i, [[1, 65]], (VPADC + qc - 2) * 130)
                              if "B" in _VAR else
                              ap_of(v2i, [[2, 65]], (VPADC + qc - 2) * 130 + hl))
                        nc.tensor.matmul(
                            od[0:64, hl, :], ptall[:, hl, qc - 2, 256:W], mv,
                            start=False, stop=True,
                            skip_group_check=True,
                        )
                od_tiles[qc] = od

            def emit_tail(qc):
                od = od_tiles.pop(qc)
                zt = sbf.tile([128, 2], f32, tag="zt", name="zt")
                nc.vector.tensor_add(
                    zt, ap_of(od, [[65, 2]], 64), zdy[:, qc, :]
                )
                rzt = sbf.tile([128, 2], f32, tag="rzt", name="rzt")
                nc.vector.reciprocal(rzt, zt)
                a_t = sbf.tile([128, 128], bf, tag="a", name="a")
                av = ap_of(a_t, [[2, 64], [1, 2]])
                nc.vector.tensor_add(
                    av,
                    ap_of(od, [[1, 64], [65, 2]]),
                    ap_of(dyA, [[2, 64], [1, 2]], qc * 128),
                )
                nc.gpsimd.tensor_add(a_t, a_t, dyB[:, qc, :])
                for hl in range(2):
                    nc.vector.tensor_scalar_mul(
                        ap_of(a_t, [[2, 64]], hl),
                        ap_of(a_t, [[2, 64]], hl),
                        rzt[:, hl:hl + 1],
                    )
                tp2 = psF.tile([128, 128], bf, tag="tp2", name="tp2")
                nc.tensor.transpose(tp2, a_t, ident)
                nc.vector.tensor_mul(fgT[:, qc, :], tp2, gt[:, ds(qc * 128, 128)])
                for jb in range(2):
                    po = psQ.tile([128, 512], f32, tag="po", name="po")
                    nc.tensor.matmul(
                        po, fgT[:, qc, :], wo_s[:, ds(jb * 512, 512)],
                        start=True, stop=True,
                    )
                    osl = outsb[:, qc, ds(jb * 512, 512)]
                    if jb == 0:
                        nc.scalar.copy(osl, po)
                    else:
                        nc.vector.tensor_copy(osl, po)
                if qc % 2 == 1:
                    nc.sync.dma_start(
                        out=out[:, qc - 1:qc + 1, :], in_=outsb[:, qc - 1:qc + 1, :]
                    )

            emit_od(0)
            for qc in range(NCH):
                if qc + 1 < NCH:
                    emit_od(qc + 1)
                emit_tail(qc)

        if debug:
            for nm, sb in [
                ("dqt", qt), ("dkt", kt), ("dvt", vt), ("dgt", gt),
                ("dv2", v2i), ("ddyP", dyPh), ("dzdy", zdy),
                ("ddyA", dyA), ("ddyB", dyB), ("dfgT", fgT), ("dpt", ptall),
            ]:
                if nm in dbg:
                    nc.sync.dma_start(out=dbg[nm], in_=sb)

    nc.compile()
    return nc


def _host_prep(x, Wqkv, bqkv, Wgate, bgate, Wout, bout, pos_bias):
    x2 = np.asarray(x, np.float32).reshape(N, D)
    xT = np.ascontiguousarray(
        np.ascontiguousarray(x2.T).reshape(NXC, 128, N).transpose(1, 0, 2)
    ).astype(BF16)

    psi = np.array([64 * (j % 2) + j // 2 for j in range(128)])

    def wpack(Wrows):
        # (128, NXC * 128): [p, c, m] = W[m, 128c + p]
        Wt = np.ascontiguousarray(Wrows.T).reshape(NXC, 128, 128).transpose(1, 0, 2)
        return np.ascontiguousarray(Wt).reshape(128, NXC * 128)

    scale = HD ** -0.5
    in_maps = []
    for cid in range(NCORES):
        r0 = 128 * cid
        wq = wpack(Wqkv[r0:r0 + 128] * scale)
        wk = wpack(Wqkv[D + r0:D + r0 + 128])
        wv = wpack(Wqkv[2 * D + r0:2 * D + r0 + 128][psi])
        wgp = wpack(Wgate[r0:r0 + 128][psi])
        wop = np.ascontiguousarray(Wout[:, r0:r0 + 128][:, psi].T)  # (128, 1024)

        biases = np.stack(
            [
                bqkv[r0:r0 + 128] * scale,
                bqkv[D + r0:D + r0 + 128],
                bqkv[2 * D + r0:2 * D + r0 + 128][psi],
                0.5 * bgate[r0:r0 + 128][psi],
            ],
            axis=1,
        )  # (128, 4)

        em = np.zeros((128, 2, W), np.float32)
        p = np.arange(128)[:, None]
        j = np.arange(W)[None, :]
        o = j - p
        sel = np.isin(o, BAND_OFFS)
        oc = np.clip(o, 0, 192)
        for hl in range(2):
            eb = np.array(
                [math.exp(pos_bias[OFFSETS.index(v), 2 * cid + hl])
                 if v in BAND_OFFS else 0.0 for v in range(193)],
                np.float32,
            )
            em[:, hl, :] = np.where(sel, eb[oc], 0.0)

        nidx = (np.arange(128)[:, None, None, None]
                + 128 * np.arange(NCH)[None, :, None, None])
        dys = np.array(DY6)[None, None, :, None]
        valid = (nidx >= dys).astype(np.float32)        # (128, NCH, NDY, 1)
        eb9 = np.zeros((1, 1, NDY, 2), np.float32)
        for hl in range(2):
            for i, d in enumerate(DY6):
                eb9[0, 0, i, hl] = math.exp(pos_bias[OFFSETS.index(d), 2 * cid + hl])
        v9t = valid * eb9                               # (128, NCH, NDY, 2)

        wpkm = np.concatenate(
            [
                wq, wk, wv, wgp, wop,
                em.reshape(128, -1),
                v9t.reshape(128, -1),
            ],
            axis=1,
        ).astype(BF16)
        in_maps.append({
            "xT": xT,
            "wpk": np.ascontiguousarray(wpkm),
            "bias4": np.ascontiguousarray(biases.astype(np.float32)),
        })
    return in_maps


def kernel(x, Wqkv, bqkv, Wgate, bgate, Wout, bout, pos_bias, offsets=None, **kw):
    x = np.asarray(x, np.float32)
    Wqkv = np.asarray(Wqkv, np.float32)
    bqkv = np.asarray(bqkv, np.float32)
    Wgate = np.asarray(Wgate, np.float32)
    bgate = np.asarray(bgate, np.float32)
    Wout = np.asarray(Wout, np.float32)
    bout = np.asarray(bout, np.float32)
    pos_bias = np.asarray(pos_bias, np.float32)

    if "nc" not in _STATE:
        _STATE["nc"] = _build_nc()
    nc = _STATE["nc"]

    in_maps = _host_prep(x, Wqkv, bqkv, Wgate, bgate, Wout, bout, pos_bias)

    from concourse.bass_utils import run_bass_kernel_spmd

    res = run_bass_kernel_spmd(
        nc, in_maps, core_ids=list(range(NCORES)), **_STATE.get("run_kwargs", {})
    )
    _STATE["last"] = res

    total = np.zeros((N, D), np.float32)
    for r in res.results:
        total += r["out"].astype(np.float32).transpose(1, 0, 2).reshape(N, D)
    total += bout
    return total.reshape(1, N, D).astype(np.float32)
